# revision 17
# baseline (speedup 1.0000x reference)
"""MoE layer (8 experts, top-2, SwiGLU) for Trainium2, expert-parallel over 8 cores.

Strategy:
  - Router (x @ router_w, top-2, softmax) runs on host in fp32 — it is 0.01%
    of the FLOPs and determines the (data-dependent) sharding.
  - Each core is assigned one expert. Tokens routed to that expert are
    gathered on host, padded to capacity C, and shipped transposed as
    xt [D, C] so the GEMMs need no on-device transpose:
        h1T = w1.T @ x.T   (lhsT = w1 chunk [128, Hp-slice], rhs = xt)
        h2T = w2.T @ xw.T  (xw = combine_weight * x, prescaled on host —
                            folding the routing weight into the mm2 operand
                            makes the SwiGLU product wv*silu(h1)*h2 directly,
                            so no on-device scaling pass is needed)
        hT  = silu(h1T) * h2T                       [Hp, C] bf16
        yT  = w3.T-chunks @ hT  (lhsT = w3 chunk [128 hid, 128 outD],
                            rhs = hT block) — w3 is the stationary operand,
                            so mm3 streams exactly C token-rows (no
                            ceil-to-128 padding waste on the token dim).
    Output ships transposed as yt [8, 128, C] fp32; host re-transposes and
    scatter-adds the 8 per-expert outputs back to [B,S,D].

  Matmuls run in bf16 (fp32 accumulate in PSUM); hidden dim 2730 is padded
  to 2816 = 22*128 (zero pad is exact: silu(0)*0 = 0).

  PSUM budget: ps1/ps2 [128,512] double-buffered (4 banks) + 2 yT
  accumulators [128,512] double-buffered (4 banks) = 8 banks; mm3 runs in
  four passes of 2 output-D chunks, alternating bank pairs so each pass's
  PSUM->SBUF copy overlaps the next pass's accumulation.
"""

import os

import numpy as np
import ml_dtypes

DIM = 1024
NUM_EXPERTS = 8
HIDDEN = 2730
P = 128
HP = 2816  # hidden padded to 22*128
KD = DIM // P  # 8 contraction chunks for mm1/mm2
HPT = HP // P  # 22 chunks of the hidden dim
NBLK = 512  # token block (moving free dim per matmul)
DCH = DIM // P  # 8 output-D chunks for mm3
DPASS = 2  # outD chunks per mm3 pass (2 PSUM banks, double-buffered)

TRACE = os.environ.get("MOE_TRACE", "0") == "1"
LAST_RESULT = None  # BassKernelResults of the last run (for test harness)

_KERNELS: dict = {}


def _build(C: int, c_real: int | None = None, reps: int = 1, unroll: int = 1,
           stream: bool = True):
    """Build + compile the per-core Bass kernel for capacity C (multiple of 128).

    reps > 1 wraps the streaming part (x/xw DMA + compute + y DMA) in a
    device-side loop; the expert weights are DMA'd once outside the loop
    (weights-resident steady state) — used only for wall-clock benchmarking.
    unroll: bodies per loop iteration (reps must divide); reduces the
    per-iteration all-engine-barrier overhead.
    stream=False: benchmark-only variant that keeps x/xw resident (block 0's
    tiles reused for every block) and writes y only on the last block —
    isolates PE + loop overhead from per-rep DMA.
    """
    import concourse.mybir as mybir
    import concourse.tile as tile
    from concourse import bacc

    dt = mybir.dt
    nc = bacc.Bacc(None, target_bir_lowering=False)

    xt = nc.dram_tensor("xt", [KD, P, C], dt.bfloat16, kind="ExternalInput")
    xw = nc.dram_tensor("xw", [KD, P, C], dt.bfloat16, kind="ExternalInput")
    w1 = nc.dram_tensor("w1", [KD, P, HP], dt.bfloat16, kind="ExternalInput")
    w2 = nc.dram_tensor("w2", [KD, P, HP], dt.bfloat16, kind="ExternalInput")
    w3 = nc.dram_tensor("w3", [HPT, P, DIM], dt.bfloat16, kind="ExternalInput")
    yt = nc.dram_tensor("yt", [DCH, P, C], dt.float32, kind="ExternalOutput")

    # Only c_real tokens are real; rows beyond that are padding whose
    # output the host ignores, so the last block shrinks to the real count.
    if c_real is None:
        c_real = C
    blocks = []
    c0 = 0
    while c0 < c_real:
        bn = min(NBLK, c_real - c0)
        blocks.append((c0, bn))
        c0 += bn

    with tile.TileContext(nc) as tc:
        with (
            tc.tile_pool(name="wpool", bufs=1) as wpool,
            tc.tile_pool(name="xpool", bufs=2) as xpool,
            tc.tile_pool(name="xwpool", bufs=2) as xwpool,
            tc.tile_pool(name="hpool", bufs=1) as hpool,
            tc.tile_pool(name="tpool", bufs=2) as tpool,
            tc.tile_pool(name="ypool", bufs=3) as ypool,
            tc.tile_pool(name="psA", bufs=2, space="PSUM") as psA,
            tc.tile_pool(name="psB", bufs=2, space="PSUM") as psB,
            tc.tile_pool(name="psC", bufs=2, space="PSUM") as psC,
        ):
            # Resident weights, DMA'd in hp-sliced parts in the order the
            # first block's matmuls consume them, split across the SP HWDGE
            # queue (w1, w3) and the gpsimd SWDGE queue (w2).
            w1_sb = [
                wpool.tile([P, HP], dt.bfloat16, name=f"w1_{kd}", tag=f"w1_{kd}")
                for kd in range(KD)
            ]
            w2_sb = [
                wpool.tile([P, HP], dt.bfloat16, name=f"w2_{kd}", tag=f"w2_{kd}")
                for kd in range(KD)
            ]
            w3_sb = [
                wpool.tile([P, DIM], dt.bfloat16, name=f"w3_{hp}", tag=f"w3_{hp}")
                for hp in range(HPT)
            ]

            def load_weights():
                bounds = [0, 3 * P, 7 * P, 12 * P, 17 * P, HP]
                for pi in range(len(bounds) - 1):
                    sl = slice(bounds[pi], bounds[pi + 1])
                    for kd in range(KD):
                        nc.sync.dma_start(w1_sb[kd][:, sl], w1[kd][:, sl])
                    for kd in range(KD):
                        nc.gpsimd.dma_start(w2_sb[kd][:, sl], w2[kd][:, sl])
                for hp in range(HPT):
                    nc.sync.dma_start(w3_sb[hp][:], w3[hp])

            def make_x(pool, src, bj, c0, bn, pfx, dma):
                t = pool.tile(
                    [P, KD, NBLK], dt.bfloat16, name=f"{pfx}_{bj}", tag=f"{pfx}"
                )
                for kd in range(KD):
                    dma(t[:, kd, :bn], src[kd][:, c0 : c0 + bn])
                return t

            def emit_body(x0=None):
                for bj, (c0, bn) in enumerate(blocks):
                    if x0 is not None and (bj == 0 or not stream):
                        x_sb, xw_sb = x0
                    else:
                        x_sb = make_x(
                            xpool, xt, bj, c0, bn, "x", nc.sync.dma_start
                        )
                        xw_sb = make_x(
                            xwpool, xw, bj, c0, bn, "xw", nc.scalar.dma_start
                        )

                    # hT = silu(w1.T @ xT) * (w2.T @ xwT), one hp chunk at a time
                    h_sbs = []
                    for hp in range(HPT):
                        ps1 = psA.tile(
                            [P, bn], dt.float32, name=f"ps1_{bj}_{hp}", tag="ps1"
                        )
                        for kd in range(KD):
                            nc.tensor.matmul(
                                ps1[:],
                                w1_sb[kd][:, hp * P : (hp + 1) * P],
                                x_sb[:, kd, 0:bn],
                                start=(kd == 0),
                                stop=(kd == KD - 1),
                            )
                        # bf16 silu halves ACT's SBUF write traffic (PE read
                        # port pressure); h is rounded to bf16 right after
                        # anyway, so the extra rounding is negligible.
                        sil = tpool.tile(
                            [P, bn], dt.bfloat16, name=f"sil_{bj}_{hp}", tag="sil"
                        )
                        nc.scalar.activation(
                            sil[:], ps1[:], mybir.ActivationFunctionType.Silu
                        )
                        ps2 = psB.tile(
                            [P, bn], dt.float32, name=f"ps2_{bj}_{hp}", tag="ps2"
                        )
                        for kd in range(KD):
                            nc.tensor.matmul(
                                ps2[:],
                                w2_sb[kd][:, hp * P : (hp + 1) * P],
                                xw_sb[:, kd, 0:bn],
                                start=(kd == 0),
                                stop=(kd == KD - 1),
                            )
                        ht = hpool.tile(
                            [P, bn], dt.bfloat16, name=f"h_{bj}_{hp}", tag=f"h_{hp}"
                        )
                        nc.vector.tensor_mul(ht[:], sil[:], ps2[:])
                        h_sbs.append(ht)

                    # yT[dc] = sum_hp w3[hp, dc].T @ hT[hp]  (w3 stationary)
                    for ps in range(DCH // DPASS):
                        ys = [
                            psC.tile(
                                [P, bn],
                                dt.float32,
                                name=f"ys_{bj}_{ps}_{dj}",
                                tag=f"ys_{dj}",
                            )
                            for dj in range(DPASS)
                        ]
                        for hp in range(HPT):
                            for dj in range(DPASS):
                                dc = ps * DPASS + dj
                                nc.tensor.matmul(
                                    ys[dj][:],
                                    w3_sb[hp][:, dc * P : (dc + 1) * P],
                                    h_sbs[hp][:, 0:bn],
                                    start=(hp == 0),
                                    stop=(hp == HPT - 1),
                                )
                        for dj in range(DPASS):
                            dc = ps * DPASS + dj
                            yo = ypool.tile(
                                [P, bn], dt.float32, name=f"y_{bj}_{dc}", tag="yo"
                            )
                            # split the PSUM->SBUF copy across ACT and DVE so
                            # the bank frees ~2x sooner (the next mm3 pass
                            # reuses it)
                            hn = bn // 2
                            nc.scalar.copy(yo[:, :hn], ys[dj][:, :hn])
                            nc.vector.tensor_copy(yo[:, hn:bn], ys[dj][:, hn:bn])
                            if stream or bj == len(blocks) - 1:
                                nc.sync.dma_start(
                                    yt[dc][:, c0 : c0 + bn], yo[:]
                                )

            if reps > 1:
                load_weights()
                x0 = None
                if not stream:
                    c0, bn = blocks[0]
                    x0 = (
                        make_x(xpool, xt, 0, c0, bn, "x", nc.sync.dma_start),
                        make_x(
                            xwpool, xw, 0, c0, bn, "xw", nc.scalar.dma_start
                        ),
                    )
                assert reps % unroll == 0
                with tc.For_i(0, reps // unroll, 1):
                    for _ in range(unroll):
                        emit_body(x0)
            else:
                # First block's activations first, so mm1 can start before
                # the 17MB weight load completes.
                c0, bn = blocks[0]
                x0 = (
                    make_x(xpool, xt, 0, c0, bn, "x", nc.sync.dma_start),
                    make_x(xwpool, xw, 0, c0, bn, "xw", nc.scalar.dma_start),
                )
                load_weights()
                emit_body(x0)

    nc.compile()
    return nc


def _route(xf: np.ndarray, router_w: np.ndarray):
    """Top-2 routing + softmax weights, fp32, matching the jax reference."""
    T = xf.shape[0]
    logits = xf @ router_w  # [T, E]
    rows = np.arange(T)
    i1 = logits.argmax(axis=1)
    tmp = logits.copy()
    tmp[rows, i1] = -np.inf
    i2 = tmp.argmax(axis=1)
    v1 = logits[rows, i1]
    v2 = tmp[rows, i2]
    e2 = np.exp((v2 - v1).astype(np.float32))
    g1 = 1.0 / (1.0 + e2)
    g2 = e2 / (1.0 + e2)
    return i1, i2, g1.astype(np.float32), g2.astype(np.float32)


def _prepare(x, router_w, w1, w2, w3):
    """Route + dispatch on host; returns (C, in_maps, idxs, shape)."""
    x = np.asarray(x, dtype=np.float32)
    router_w = np.asarray(router_w, dtype=np.float32)
    w1 = np.asarray(w1, dtype=np.float32)
    w2 = np.asarray(w2, dtype=np.float32)
    w3 = np.asarray(w3, dtype=np.float32)

    B, S, D = x.shape
    T = B * S
    xf = x.reshape(T, D)

    i1, i2, g1, g2 = _route(xf, router_w)

    # per-expert token lists (slot-0 tokens then slot-1 tokens)
    idxs, wgts = [], []
    for e in range(NUM_EXPERTS):
        s0 = np.nonzero(i1 == e)[0]
        s1 = np.nonzero(i2 == e)[0]
        idxs.append(np.concatenate([s0, s1]))
        wgts.append(np.concatenate([g1[s0], g2[s1]]))
    max_cnt = max(len(ix) for ix in idxs)
    C = max(P, ((max_cnt + P - 1) // P) * P)

    bf16 = ml_dtypes.bfloat16
    # expert weights, padded along the hidden dim and cast to bf16
    w1p = np.zeros((NUM_EXPERTS, D, HP), dtype=bf16)
    w1p[:, :, :HIDDEN] = w1
    w2p = np.zeros((NUM_EXPERTS, D, HP), dtype=bf16)
    w2p[:, :, :HIDDEN] = w2
    w3p = np.zeros((NUM_EXPERTS, HP, D), dtype=bf16)
    w3p[:, :HIDDEN, :] = w3

    in_maps = []
    for e in range(NUM_EXPERTS):
        ix = idxs[e]
        xg = np.zeros((C, D), dtype=np.float32)
        xg[: len(ix)] = xf[ix]
        xgw = np.zeros((C, D), dtype=np.float32)
        xgw[: len(ix)] = xf[ix] * wgts[e][:, None]
        in_maps.append(
            {
                "xt": np.ascontiguousarray(xg.T).astype(bf16).reshape(KD, P, C),
                "xw": np.ascontiguousarray(xgw.T).astype(bf16).reshape(KD, P, C),
                "w1": w1p[e].reshape(KD, P, HP),
                "w2": w2p[e].reshape(KD, P, HP),
                "w3": w3p[e].reshape(HPT, P, DIM),
            }
        )
    return C, in_maps, idxs, (B, S, D)


def kernel(x, router_w, w1, w2, w3):
    global LAST_RESULT
    from concourse.bass_utils import run_bass_kernel_spmd

    C, in_maps, idxs, (B, S, D) = _prepare(x, router_w, w1, w2, w3)

    max_cnt = max(len(ix) for ix in idxs)
    key = (C, max_cnt)
    if key not in _KERNELS:
        _KERNELS[key] = _build(C, c_real=max_cnt)
    nc = _KERNELS[key]

    res = run_bass_kernel_spmd(
        nc,
        in_maps,
        list(range(NUM_EXPERTS)),
        trace=TRACE,
    )
    LAST_RESULT = res

    out = np.zeros((B * S, D), dtype=np.float32)
    for e in range(NUM_EXPERTS):
        ix = idxs[e]
        ye = res.results[e]["yt"].reshape(DIM, C).T  # [C, DIM] fp32
        out[ix] += ye[: len(ix)]
    return out.reshape(B, S, D)
